# revision 3
# baseline (speedup 1.0000x reference)
"""Trainium2 8-core kernel for LN + RoPE multi-head attention + out-proj.

Sharding: tensor-parallel over heads. Core c owns heads (2c, 2c+1) = inner dims
[128c, 128c+128). Each core computes LN(x) @ its w_qkv column-slice, RoPE,
full-sequence attention for its 2 heads, and a partial out-projection against
its w_out row-slice. Host sums the 8 partial outputs and adds b_out.

Math notes:
- ln_gamma is folded into w_qkv on the host (w_eff = gamma[:,None] * w_qkv);
  ln_beta contributes beta @ w_qkv which is zero for this problem (beta = 0).
- RoPE is computed as q' = q*cos128 + q_swap*sin128s where q_swap comes from an
  extra matmul against half-rotated weight columns (avoids cross-partition ops)
  and sin128s carries the rotation signs.
- Softmax denominators come from a constant ones-column appended to v, so the
  attention matmul accumulates sum(exp) in psum row 64 for free; normalization
  happens on the small per-head output, broadcast across partitions via a tiny
  K=2 matmul.

v2 performance structure:
- Phase B is software-pipelined around the ACT(exp) stream, which is the
  per-iteration floor ((172+1024)/1.2GHz ~ 1.1us per 128x1024 fp32-psum tile).
  Scores pairs for jc+2 are emitted before the probs@v matmuls of jc so the PE
  queue never head-of-line blocks on an exp.
- Elementwise work is spread across DVE / ACT / GPSIMD so no engine exceeds
  the PE total: rope mul/add on gpsimd, psum->sbuf casts split ACT/DVE.
- Phase A tiles for batch b+1 are interleaved between phase-B attention loops
  of batch b, emitted before the normalize/out-proj tail so their PE work
  slots under batch b's exp stream.
"""

import os
import sys

sys.path.insert(0, "/opt/trn_rl_repo")

import numpy as np

B, N, D = 4, 2048, 1024
H, HD = 16, 64
TOK = B * N  # 8192
NCORES = 8
EPS = 1e-5

_CACHE = {}


def _np_bf16():
    import ml_dtypes

    return ml_dtypes.bfloat16


def build_bass():
    import concourse.bass as bass
    import concourse.mybir as mybir
    import concourse.tile as tile
    from concourse import bacc
    from concourse.masks import make_identity

    f32 = mybir.dt.float32
    bf16 = mybir.dt.bfloat16
    AF = mybir.ActivationFunctionType
    ALU = mybir.AluOpType

    # Force Ln and Exp to resolve to the one table set containing both, so the
    # scheduler never ping-pongs ACT table loads between LN and softmax phases.
    import concourse.hw_specs as hw_specs
    if not getattr(hw_specs, "_ln_exp_patched", False):
        _orig_gat = hw_specs.get_activation_tables

        def _patched_gat(arch):
            tabs = _orig_gat(arch)
            AFt = mybir.ActivationFunctionType
            for name, funcs in tabs.items():
                if name != "natural_log_exp_and_others":
                    funcs.discard(AFt.Exp)
                    funcs.discard(AFt.Ln)
            return tabs

        hw_specs.get_activation_tables = _patched_gat
        import concourse.bacc as _bacc_mod
        _bacc_mod.get_activation_tables = _patched_gat
        hw_specs._ln_exp_patched = True

    nc = bacc.Bacc("TRN2", target_bir_lowering=False, debug=False, num_devices=NCORES)

    x_d = nc.dram_tensor("x", [TOK, D], bf16, kind="ExternalInput").ap()
    wqkv_d = nc.dram_tensor("wqkv", [D, 384], bf16, kind="ExternalInput").ap()
    perm_d = nc.dram_tensor("perm", [128, 128], bf16, kind="ExternalInput").ap()
    wout_d = nc.dram_tensor("wout", [128, D], bf16, kind="ExternalInput").ap()
    cos_d = nc.dram_tensor("cos128", [128, N], bf16, kind="ExternalInput").ap()
    sin_d = nc.dram_tensor("sin128s", [128, N], bf16, kind="ExternalInput").ap()
    e2_d = nc.dram_tensor("e2", [2, 128], bf16, kind="ExternalInput").ap()
    out_d = nc.dram_tensor("out", [TOK, D], bf16, kind="ExternalOutput").ap()

    with tile.TileContext(nc) as tc:
        with (
            tc.tile_pool(name="singles", bufs=1) as singles,
            tc.tile_pool(name="xin", bufs=2) as xin_pool,
            tc.tile_pool(name="xn", bufs=3) as xn_pool,
            tc.tile_pool(name="xnt", bufs=3) as xnt_pool,
            tc.tile_pool(name="small", bufs=6) as small,
            tc.tile_pool(name="rtmp", bufs=6) as rtmp,
            tc.tile_pool(name="vst", bufs=3) as vst_pool,
            tc.tile_pool(name="probs", bufs=3) as pr_pool,
            tc.tile_pool(name="tmph", bufs=4) as tmph_pool,
            tc.tile_pool(name="norm", bufs=2) as norm_pool,
            tc.tile_pool(name="ost", bufs=3) as ost_pool,
            tc.tile_pool(name="ob", bufs=2) as ob_pool,
            tc.tile_pool(name="psA", bufs=2, space="PSUM") as ps_a,
            tc.tile_pool(name="psS", bufs=2, space="PSUM") as ps_st,
            tc.tile_pool(name="psOS", bufs=2, space="PSUM") as ps_os_pool,
        ):
            # ---- constants / persistent tiles ----
            ident = singles.tile([128, 128], bf16)
            make_identity(nc, ident)
            eps_sb = singles.tile([128, 1], f32)
            nc.vector.memset(eps_sb, EPS)
            e2 = singles.tile([2, 128], bf16)
            nc.sync.dma_start(out=e2, in_=e2_d)

            wqkv_sb = singles.tile([128, 8, 384], bf16)
            perm_sb = singles.tile([128, 128], bf16)
            nc.sync.dma_start(out=perm_sb, in_=perm_d)
            nc.sync.dma_start(
                out=wqkv_sb, in_=wqkv_d.rearrange("(c p) f -> p c f", p=128)
            )
            wout_sb = singles.tile([128, D], bf16)
            nc.sync.dma_start(out=wout_sb, in_=wout_d)
            cos_sb = singles.tile([128, N], bf16)
            nc.sync.dma_start(out=cos_sb, in_=cos_d)
            sin_sb = singles.tile([128, N], bf16)
            nc.sync.dma_start(out=sin_sb, in_=sin_d)

            qT = singles.tile([128, TOK], bf16)  # rows: head0 dims 0-63, head1 64-127
            kT = singles.tile([128, TOK], bf16)
            v_sb = singles.tile([128, 64, 2, 65], bf16)  # [j, jchunk, head, 64v+1]
            nc.vector.memset(v_sb[:, :, :, 64:65], 1.0)

            # ---- phase A: LN -> transpose -> QKV(+swapped) -> RoPE ----
            def phase_a(tt):  # 512-token tiles
                p0 = (tt % 4) * 512  # position within batch for rope tables
                xnT = xnt_pool.tile([128, 8, 512], bf16)
                x_t4 = xin_pool.tile([128, 4, D], bf16)
                nc.gpsimd.dma_start(
                    out=x_t4,
                    in_=x_d[tt * 512 : (tt + 1) * 512, :].rearrange(
                        "(t p) d -> p t d", p=128
                    ),
                )
                for st in range(4):
                    x_t = x_t4[:, st, :]
                    stats = small.tile([128, 2, 6], f32)
                    nc.vector.bn_stats(out=stats[:, 0, :], in_=x_t[:, 0:512])
                    nc.vector.bn_stats(out=stats[:, 1, :], in_=x_t[:, 512:1024])
                    mv = small.tile([128, 2], f32)
                    nc.vector.bn_aggr(out=mv, in_=stats)
                    lnv = small.tile([128, 1], f32)
                    nc.scalar.activation(lnv, mv[:, 1:2], AF.Ln, bias=eps_sb)
                    rstd = small.tile([128, 1], f32)
                    nc.scalar.activation(rstd, lnv, AF.Exp, scale=-0.5)
                    xn = xn_pool.tile([128, D], bf16)
                    nc.gpsimd.tensor_scalar(
                        out=xn,
                        in0=x_t,
                        scalar1=mv[:, 0:1],
                        scalar2=rstd,
                        op0=ALU.subtract,
                        op1=ALU.mult,
                    )
                    # 8 transposes into one psum tile, then copy out (split
                    # between ACT and DVE to balance engine load)
                    ptx = ps_a.tile([128, 1024], bf16, tag="psA")
                    for dc in range(8):
                        nc.tensor.transpose(
                            ptx[:, dc * 128 : (dc + 1) * 128],
                            xn[:, dc * 128 : (dc + 1) * 128],
                            ident,
                        )
                    nc.scalar.activation(
                        out=xnT[:, 0:4, st * 128 : st * 128 + 128],
                        in_=ptx[:, 0:512].rearrange("p (c t) -> p c t", c=4),
                        func=AF.Copy,
                    )
                    nc.vector.tensor_copy(
                        out=xnT[:, 4:8, st * 128 : st * 128 + 128],
                        in_=ptx[:, 512:1024].rearrange("p (c t) -> p c t", c=4),
                    )
                # QKV projections: f= 0:q 1:k 2:v
                for f in (0, 1, 2):
                    ps_q = ps_a.tile([128, 512], f32, tag="psA")
                    for dc in range(8):
                        nc.tensor.matmul(
                            ps_q,
                            wqkv_sb[:, dc, f * 128 : (f + 1) * 128],
                            xnT[:, dc, :],
                            start=dc == 0,
                            stop=dc == 7,
                        )
                    if f == 2:
                        # v: transpose to token-major into v_sb
                        vstage = vst_pool.tile([128, 512], bf16)
                        nc.vector.tensor_copy(out=vstage, in_=ps_q)
                        ptv = ps_a.tile([128, 512], bf16, tag="psA")
                        for st in range(4):
                            nc.tensor.transpose(
                                ptv[:, st * 128 : (st + 1) * 128],
                                vstage[:, st * 128 : (st + 1) * 128],
                                ident,
                            )
                        nc.vector.tensor_copy(
                            out=v_sb[:, tt * 4 : tt * 4 + 4, :, 0:64],
                            in_=ptv.rearrange("p (c h d) -> p c h d", c=4, h=2),
                        )
                    else:
                        # rope: qT = q*cos + (perm.T @ q)*sin_signed
                        q_sb = rtmp.tile([128, 512], bf16, tag="qsb")
                        nc.scalar.activation(out=q_sb, in_=ps_q, func=AF.Copy)
                        ps_qsw = ps_a.tile([128, 512], f32, tag="psA")
                        nc.tensor.matmul(
                            ps_qsw, perm_sb, q_sb, start=True, stop=True
                        )
                        a = rtmp.tile([128, 512], bf16, tag="ra")
                        nc.gpsimd.tensor_tensor(
                            out=a, in0=q_sb, in1=cos_sb[:, p0 : p0 + 512], op=ALU.mult
                        )
                        bt = rtmp.tile([128, 512], bf16, tag="rb")
                        nc.vector.tensor_tensor(
                            out=bt, in0=ps_qsw, in1=sin_sb[:, p0 : p0 + 512], op=ALU.mult
                        )
                        dst = qT if f == 0 else kT
                        nc.gpsimd.tensor_tensor(
                            out=dst[:, tt * 512 : (tt + 1) * 512],
                            in0=a,
                            in1=bt,
                            op=ALU.add,
                        )

            # ---- phase B: scores -> softmax -> probs@v -> normalize -> out ----
            def phase_b_it(b, it, mid_emit=None):
                i0 = b * 2048 + it * 512
                ps_os = [
                    ps_os_pool.tile([65, 512], f32, tag="psOS", name=f"ps_o_{b}_{it}_{h}")
                    for h in range(2)
                ]
                probs_tiles = {}

                def emit_scores(jc):
                    j0 = b * 2048 + jc * 128
                    ps_s = ps_st.tile([128, 1024], f32, tag="psS")
                    for h in range(2):
                        hb = h * 64
                        nc.tensor.matmul(
                            ps_s[:, h * 512 : (h + 1) * 512],
                            kT[hb : hb + 64, j0 : j0 + 128],
                            qT[hb : hb + 64, i0 : i0 + 512],
                            start=True,
                            stop=True,
                            tile_position=(hb, 0),
                        )
                    probs = pr_pool.tile([128, 1024], bf16)
                    nc.scalar.activation(probs, ps_s, AF.Exp, scale=HD**-0.5)
                    probs_tiles[jc] = probs

                def emit_v(jc):
                    jcg = b * 16 + jc
                    probs = probs_tiles.pop(jc)
                    for h in range(2):
                        nc.tensor.matmul(
                            ps_os[h],
                            v_sb[:, jcg, h, :],
                            probs[:, h * 512 : (h + 1) * 512],
                            start=jc == 0,
                            stop=jc == 15,
                        )

                PIPE = 2
                for jc in range(PIPE):
                    emit_scores(jc)
                for jc in range(16):
                    if jc + PIPE < 16:
                        emit_scores(jc + PIPE)
                    emit_v(jc)

                # interleaved phase-A work for the next batch lands here so its
                # PE/DVE stream overlaps this tile's exp tail
                if mid_emit is not None:
                    mid_emit()

                tmpA = tmph_pool.tile([65, 512], bf16, tag="tmpA")
                nc.vector.tensor_copy(out=tmpA, in_=ps_os[0])
                tmpB = tmph_pool.tile([65, 512], bf16, tag="tmpB")
                nc.vector.tensor_copy(out=tmpB, in_=ps_os[1])
                # denominators -> reciprocal -> broadcast via K=2 matmul
                rbf = norm_pool.tile([2, 512], bf16, tag="rbf")
                nc.sync.dma_start(out=rbf[0:1, :], in_=tmpA[64:65, :])
                nc.sync.dma_start(out=rbf[1:2, :], in_=tmpB[64:65, :])
                rf = norm_pool.tile([2, 512], f32, tag="rf")
                nc.vector.tensor_copy(out=rf, in_=rbf)
                nc.vector.reciprocal(out=rf, in_=rf)
                rbf2 = norm_pool.tile([2, 512], bf16, tag="rbf2")
                nc.vector.tensor_copy(out=rbf2, in_=rf)
                ps_bc = ps_st.tile([128, 512], f32, tag="psS")
                nc.tensor.matmul(ps_bc, e2, rbf2, start=True, stop=True)
                bc = norm_pool.tile([128, 512], bf16, tag="bc")
                nc.scalar.activation(out=bc, in_=ps_bc, func=AF.Copy)
                ostack = ost_pool.tile([128, 512], bf16)
                nc.vector.tensor_tensor(
                    out=ostack[0:64, :], in0=tmpA[0:64, :], in1=bc[0:64, :], op=ALU.mult
                )
                nc.sync.dma_start(out=ostack[64:128, :], in_=tmpB[0:64, :])
                nc.vector.tensor_tensor(
                    out=ostack[64:128, :],
                    in0=ostack[64:128, :],
                    in1=bc[64:128, :],
                    op=ALU.mult,
                )
                # out-projection for these 512 tokens, staged then one DMA
                ob_big = ob_pool.tile([128, 4, D], bf16)
                for t4 in range(4):
                    for Dc in range(2):
                        ps_op = ps_st.tile([128, 512], f32, tag="psS")
                        nc.tensor.matmul(
                            ps_op,
                            ostack[:, t4 * 128 : (t4 + 1) * 128],
                            wout_sb[:, Dc * 512 : (Dc + 1) * 512],
                            start=True,
                            stop=True,
                        )
                        nc.vector.tensor_copy(
                            out=ob_big[:, t4, Dc * 512 : (Dc + 1) * 512], in_=ps_op
                        )
                nc.sync.dma_start(
                    out=out_d[i0 : i0 + 512, :].rearrange(
                        "(t p) d -> p t d", p=128
                    ),
                    in_=ob_big,
                )

            for tt in range(4):
                phase_a(tt)
            for b in range(4):
                for it in range(4):
                    if b < 3:
                        tt_next = 4 * (b + 1) + it
                        phase_b_it(b, it, mid_emit=lambda tt=tt_next: phase_a(tt))
                    else:
                        phase_b_it(b, it)

    nc.finalize()
    return nc


def make_in_maps(x, ln_gamma, ln_beta, w_qkv):
    bf = _np_bf16()
    x = np.asarray(x, np.float32).reshape(TOK, D).astype(bf)
    g = np.asarray(ln_gamma, np.float32)
    w = np.asarray(w_qkv, np.float32)
    w_eff = g[:, None] * w  # [D, 3*INNER]

    # rope tables
    inv_freq = 1.0 / (10000.0 ** (np.arange(0, HD, 2, dtype=np.float32) / HD))
    pos = np.arange(N, dtype=np.float32)
    ang = pos[:, None] * inv_freq[None, :]  # [N, 32]
    cosT = np.cos(ang).T.astype(np.float32)  # [32, N]
    sinT = np.sin(ang).T.astype(np.float32)
    cos128 = np.tile(cosT, (4, 1)).astype(bf)  # rows p -> cos[p%32]
    sin128s = np.tile(sinT, (4, 1)).astype(np.float32)
    sin128s[0:32] *= -1.0
    sin128s[64:96] *= -1.0
    sin128s = sin128s.astype(bf)

    perm_np = np.zeros((128, 128), np.float32)
    for p in range(128):
        sig = (p % 64 + 32) % 64 + 64 * (p // 64)
        perm_np[sig, p] = 1.0
    perm_np = perm_np.astype(bf)

    e2_np = np.zeros((2, 128), np.float32)
    e2_np[0, 0:64] = 1.0
    e2_np[1, 64:128] = 1.0
    e2_np = e2_np.astype(bf)

    in_maps = []
    for c in range(NCORES):
        sl = slice(128 * c, 128 * c + 128)
        wq = w_eff[:, 0:1024][:, sl]
        wk = w_eff[:, 1024:2048][:, sl]
        wv = w_eff[:, 2048:3072][:, sl]

        def swap_halves(m):
            m4 = m.reshape(D, 2, 2, 32)
            return m4[:, :, ::-1, :].reshape(D, 128)

        wcat = np.concatenate([wq, wk, wv], axis=1).astype(bf)
        in_maps.append(
            {
                "x": x,
                "wqkv": np.ascontiguousarray(wcat),
                "wout": None,  # filled below by caller (needs w_out)
                "cos128": cos128,
                "sin128s": sin128s,
                "e2": e2_np,
                "perm": perm_np,
            }
        )
    return in_maps


def _run(inputs, trace=False):
    from concourse import bass_utils

    if "nc" not in _CACHE:
        _CACHE["nc"] = build_bass()
    nc = _CACHE["nc"]

    bf = _np_bf16()
    x = inputs["x"]
    w_out = np.asarray(inputs["w_out"], np.float32)
    b_out = np.asarray(inputs["b_out"], np.float32)
    beta = np.asarray(inputs["ln_beta"], np.float32)
    assert np.allclose(beta, 0.0, atol=1e-12), "nonzero ln_beta unsupported"

    in_maps = make_in_maps(
        inputs["x"], inputs["ln_gamma"], inputs["ln_beta"], inputs["w_qkv"]
    )
    for c in range(NCORES):
        in_maps[c]["wout"] = np.ascontiguousarray(
            w_out[128 * c : 128 * c + 128, :].astype(bf)
        )

    res = bass_utils.run_bass_kernel_spmd(
        nc, in_maps, core_ids=list(range(NCORES)), trace=trace
    )
    total = np.zeros((TOK, D), np.float32)
    for r in res.results:
        total += np.asarray(r["out"], np.float32)
    total += b_out[None, :]
    return total.reshape(B, N, D), res


def kernel(**inputs):
    out, _ = _run(inputs, trace=False)
    return out


# revision 5
# speedup vs baseline: 2.3530x; 2.3530x over previous
"""Trainium2 8-core kernel for LN + RoPE multi-head attention + out-proj.

Sharding: tensor-parallel over heads. Core c owns heads (2c, 2c+1) = inner dims
[128c, 128c+128). Each core computes LN(x) @ its w_qkv column-slice, RoPE,
full-sequence attention for its 2 heads, and a partial out-projection against
its w_out row-slice. Host sums the 8 partial outputs and adds b_out.

Math notes:
- ln_gamma is folded into w_qkv on the host (w_eff = gamma[:,None] * w_qkv);
  ln_beta contributes beta @ w_qkv which is zero for this problem (beta = 0).
- RoPE is computed as q' = q*cos128 + q_swap*sin128s where q_swap comes from an
  extra matmul against half-rotated weight columns (avoids cross-partition ops)
  and sin128s carries the rotation signs.
- Softmax denominators come from a constant ones-column appended to v, so the
  attention matmul accumulates sum(exp) in psum row 64 for free; normalization
  happens on the small per-head output, broadcast across partitions via a tiny
  K=2 matmul.

v2 performance structure:
- Phase B is software-pipelined around the ACT(exp) stream, which is the
  per-iteration floor ((172+1024)/1.2GHz ~ 1.1us per 128x1024 fp32-psum tile).
  Scores pairs for jc+2 are emitted before the probs@v matmuls of jc so the PE
  queue never head-of-line blocks on an exp.
- Elementwise work is spread across DVE / ACT / GPSIMD so no engine exceeds
  the PE total: rope mul/add on gpsimd, psum->sbuf casts split ACT/DVE.
- Phase A tiles for batch b+1 are interleaved between phase-B attention loops
  of batch b, emitted before the normalize/out-proj tail so their PE work
  slots under batch b's exp stream.
"""

import os
import sys

sys.path.insert(0, "/opt/trn_rl_repo")

import numpy as np

B, N, D = 4, 2048, 1024
H, HD = 16, 64
TOK = B * N  # 8192
NCORES = 8
EPS = 1e-5

_CACHE = {}


def _np_bf16():
    import ml_dtypes

    return ml_dtypes.bfloat16


def build_bass():
    import concourse.bass as bass
    import concourse.mybir as mybir
    import concourse.tile as tile
    from concourse import bacc
    from concourse.masks import make_identity

    f32 = mybir.dt.float32
    bf16 = mybir.dt.bfloat16
    AF = mybir.ActivationFunctionType
    ALU = mybir.AluOpType

    # Force Ln and Exp to resolve to the one table set containing both, so the
    # scheduler never ping-pongs ACT table loads between LN and softmax phases.
    import concourse.hw_specs as hw_specs
    if not getattr(hw_specs, "_ln_exp_patched", False):
        _orig_gat = hw_specs.get_activation_tables

        def _patched_gat(arch):
            tabs = _orig_gat(arch)
            AFt = mybir.ActivationFunctionType
            for name, funcs in tabs.items():
                if name != "natural_log_exp_and_others":
                    funcs.discard(AFt.Exp)
                    funcs.discard(AFt.Ln)
            return tabs

        hw_specs.get_activation_tables = _patched_gat
        import concourse.bacc as _bacc_mod
        _bacc_mod.get_activation_tables = _patched_gat
        hw_specs._ln_exp_patched = True

    nc = bacc.Bacc("TRN2", target_bir_lowering=False, debug=False, num_devices=NCORES)

    x_d = nc.dram_tensor("x", [TOK, D], bf16, kind="ExternalInput").ap()
    wqkv_d = nc.dram_tensor("wqkv", [D, 384], bf16, kind="ExternalInput").ap()
    perm_d = nc.dram_tensor("perm", [128, 128], bf16, kind="ExternalInput").ap()
    wout_d = nc.dram_tensor("wout", [128, D], bf16, kind="ExternalInput").ap()
    cos_d = nc.dram_tensor("cos128", [128, N], bf16, kind="ExternalInput").ap()
    sin_d = nc.dram_tensor("sin128s", [128, N], bf16, kind="ExternalInput").ap()
    e2_d = nc.dram_tensor("e2", [2, 128], bf16, kind="ExternalInput").ap()
    out_d = nc.dram_tensor("out", [TOK, D], bf16, kind="ExternalOutput").ap()

    with tile.TileContext(nc) as tc:
        with (
            tc.tile_pool(name="singles", bufs=1) as singles,
            tc.tile_pool(name="xin", bufs=2) as xin_pool,
            tc.tile_pool(name="xn", bufs=3) as xn_pool,
            tc.tile_pool(name="xnt", bufs=3) as xnt_pool,
            tc.tile_pool(name="small", bufs=6) as small,
            tc.tile_pool(name="rtmp", bufs=6) as rtmp,
            tc.tile_pool(name="vst", bufs=3) as vst_pool,
            tc.tile_pool(name="probs", bufs=3) as pr_pool,
            tc.tile_pool(name="tmph", bufs=4) as tmph_pool,
            tc.tile_pool(name="norm", bufs=2) as norm_pool,
            tc.tile_pool(name="ost", bufs=3) as ost_pool,
            tc.tile_pool(name="ob", bufs=2) as ob_pool,
            tc.tile_pool(name="psA", bufs=2, space="PSUM") as ps_a,
            tc.tile_pool(name="psS", bufs=2, space="PSUM") as ps_st,
            tc.tile_pool(name="psOS", bufs=2, space="PSUM") as ps_os_pool,
        ):
            # ---- constants / persistent tiles ----
            ident = singles.tile([128, 128], bf16)
            make_identity(nc, ident)
            eps_sb = singles.tile([128, 1], f32)
            nc.vector.memset(eps_sb, EPS)
            e2 = singles.tile([2, 128], bf16)
            nc.sync.dma_start(out=e2, in_=e2_d)

            wqkv_sb = singles.tile([128, 8, 384], bf16)
            perm_sb = singles.tile([128, 128], bf16)
            nc.sync.dma_start(out=perm_sb, in_=perm_d)
            nc.sync.dma_start(
                out=wqkv_sb, in_=wqkv_d.rearrange("(c p) f -> p c f", p=128)
            )
            wout_sb = singles.tile([128, D], bf16)
            nc.sync.dma_start(out=wout_sb, in_=wout_d)
            cos_sb = singles.tile([128, N], bf16)
            nc.sync.dma_start(out=cos_sb, in_=cos_d)
            sin_sb = singles.tile([128, N], bf16)
            nc.sync.dma_start(out=sin_sb, in_=sin_d)

            qT = singles.tile([128, TOK], bf16)  # rows: head0 dims 0-63, head1 64-127
            kT = singles.tile([128, TOK], bf16)
            v_sb = singles.tile([128, 64, 2, 65], bf16)  # [j, jchunk, head, 64v+1]
            nc.vector.memset(v_sb[:, :, :, 64:65], 1.0)

            # ---- phase A: LN -> transpose -> QKV(+swapped) -> RoPE ----
            def phase_a(tt):  # 512-token tiles
                p0 = (tt % 4) * 512  # position within batch for rope tables
                xnT = xnt_pool.tile([128, 8, 512], bf16)
                x_t4 = xin_pool.tile([128, 4, D], bf16)
                nc.gpsimd.dma_start(
                    out=x_t4,
                    in_=x_d[tt * 512 : (tt + 1) * 512, :].rearrange(
                        "(t p) d -> p t d", p=128
                    ),
                )
                for st in range(4):
                    x_t = x_t4[:, st, :]
                    stats = small.tile([128, 2, 6], f32)
                    nc.vector.bn_stats(out=stats[:, 0, :], in_=x_t[:, 0:512])
                    nc.vector.bn_stats(out=stats[:, 1, :], in_=x_t[:, 512:1024])
                    mv = small.tile([128, 2], f32)
                    nc.vector.bn_aggr(out=mv, in_=stats)
                    lnv = small.tile([128, 1], f32)
                    nc.scalar.activation(lnv, mv[:, 1:2], AF.Ln, bias=eps_sb)
                    rstd = small.tile([128, 1], f32)
                    nc.scalar.activation(rstd, lnv, AF.Exp, scale=-0.5)
                    xn = xn_pool.tile([128, D], bf16)
                    nc.vector.tensor_scalar(
                        out=xn,
                        in0=x_t,
                        scalar1=mv[:, 0:1],
                        scalar2=rstd,
                        op0=ALU.subtract,
                        op1=ALU.mult,
                    )
                    # 8 transposes into one psum tile, one strided copy out
                    ptx = ps_a.tile([128, 1024], bf16, tag="psA")
                    for dc in range(8):
                        nc.tensor.transpose(
                            ptx[:, dc * 128 : (dc + 1) * 128],
                            xn[:, dc * 128 : (dc + 1) * 128],
                            ident,
                        )
                    nc.vector.tensor_copy(
                        out=xnT[:, :, st * 128 : st * 128 + 128],
                        in_=ptx.rearrange("p (c t) -> p c t", c=8),
                    )
                # QKV projections: f= 0:q 1:k 2:v
                for f in (0, 1, 2):
                    ps_q = ps_a.tile([128, 512], f32, tag="psA")
                    for dc in range(8):
                        nc.tensor.matmul(
                            ps_q,
                            wqkv_sb[:, dc, f * 128 : (f + 1) * 128],
                            xnT[:, dc, :],
                            start=dc == 0,
                            stop=dc == 7,
                        )
                    if f == 2:
                        # v: transpose to token-major into v_sb
                        vstage = vst_pool.tile([128, 512], bf16)
                        nc.vector.tensor_copy(out=vstage, in_=ps_q)
                        ptv = ps_a.tile([128, 512], bf16, tag="psA")
                        for st in range(4):
                            nc.tensor.transpose(
                                ptv[:, st * 128 : (st + 1) * 128],
                                vstage[:, st * 128 : (st + 1) * 128],
                                ident,
                            )
                        nc.vector.tensor_copy(
                            out=v_sb[:, tt * 4 : tt * 4 + 4, :, 0:64],
                            in_=ptv.rearrange("p (c h d) -> p c h d", c=4, h=2),
                        )
                    else:
                        # rope: qT = q*cos + (perm.T @ q)*sin_signed
                        q_sb = rtmp.tile([128, 512], bf16, tag="qsb")
                        nc.scalar.activation(out=q_sb, in_=ps_q, func=AF.Copy)
                        ps_qsw = ps_a.tile([128, 512], f32, tag="psA")
                        nc.tensor.matmul(
                            ps_qsw, perm_sb, q_sb, start=True, stop=True
                        )
                        a = rtmp.tile([128, 512], bf16, tag="ra")
                        nc.vector.tensor_tensor(
                            out=a, in0=q_sb, in1=cos_sb[:, p0 : p0 + 512], op=ALU.mult
                        )
                        bt = rtmp.tile([128, 512], bf16, tag="rb")
                        nc.vector.tensor_tensor(
                            out=bt, in0=ps_qsw, in1=sin_sb[:, p0 : p0 + 512], op=ALU.mult
                        )
                        dst = qT if f == 0 else kT
                        nc.vector.tensor_tensor(
                            out=dst[:, tt * 512 : (tt + 1) * 512],
                            in0=a,
                            in1=bt,
                            op=ALU.add,
                        )

            # ---- phase B: scores -> softmax -> probs@v -> normalize -> out ----
            def phase_b_it(b, it, mid_emit=None):
                i0 = b * 2048 + it * 512
                ps_os = [
                    ps_os_pool.tile([65, 512], f32, tag="psOS", name=f"ps_o_{b}_{it}_{h}")
                    for h in range(2)
                ]
                probs_tiles = {}

                def emit_scores(jc):
                    j0 = b * 2048 + jc * 128
                    ps_s = ps_st.tile([128, 1024], f32, tag="psS")
                    for h in range(2):
                        hb = h * 64
                        nc.tensor.matmul(
                            ps_s[:, h * 512 : (h + 1) * 512],
                            kT[hb : hb + 64, j0 : j0 + 128],
                            qT[hb : hb + 64, i0 : i0 + 512],
                            start=True,
                            stop=True,
                            tile_position=(hb, 0),
                        )
                    probs = pr_pool.tile([128, 1024], bf16)
                    nc.scalar.activation(probs, ps_s, AF.Exp, scale=HD**-0.5)
                    probs_tiles[jc] = probs

                def emit_v(jc):
                    jcg = b * 16 + jc
                    probs = probs_tiles.pop(jc)
                    for h in range(2):
                        nc.tensor.matmul(
                            ps_os[h],
                            v_sb[:, jcg, h, :],
                            probs[:, h * 512 : (h + 1) * 512],
                            start=jc == 0,
                            stop=jc == 15,
                        )

                PIPE = 2
                for jc in range(PIPE):
                    emit_scores(jc)
                for jc in range(16):
                    if jc + PIPE < 16:
                        emit_scores(jc + PIPE)
                    emit_v(jc)

                # interleaved phase-A work for the next batch lands here so its
                # PE/DVE stream overlaps this tile's exp tail
                if mid_emit is not None:
                    mid_emit()

                tmpA = tmph_pool.tile([65, 512], bf16, tag="tmpA")
                nc.vector.tensor_copy(out=tmpA, in_=ps_os[0])
                tmpB = tmph_pool.tile([65, 512], bf16, tag="tmpB")
                nc.vector.tensor_copy(out=tmpB, in_=ps_os[1])
                # denominators -> reciprocal -> broadcast via K=2 matmul
                rbf = norm_pool.tile([2, 512], bf16, tag="rbf")
                nc.sync.dma_start(out=rbf[0:1, :], in_=tmpA[64:65, :])
                nc.sync.dma_start(out=rbf[1:2, :], in_=tmpB[64:65, :])
                rf = norm_pool.tile([2, 512], f32, tag="rf")
                nc.vector.tensor_copy(out=rf, in_=rbf)
                nc.vector.reciprocal(out=rf, in_=rf)
                rbf2 = norm_pool.tile([2, 512], bf16, tag="rbf2")
                nc.vector.tensor_copy(out=rbf2, in_=rf)
                ps_bc = ps_st.tile([128, 512], f32, tag="psS")
                nc.tensor.matmul(ps_bc, e2, rbf2, start=True, stop=True)
                bc = norm_pool.tile([128, 512], bf16, tag="bc")
                nc.scalar.activation(out=bc, in_=ps_bc, func=AF.Copy)
                ostack = ost_pool.tile([128, 512], bf16)
                nc.vector.tensor_tensor(
                    out=ostack[0:64, :], in0=tmpA[0:64, :], in1=bc[0:64, :], op=ALU.mult
                )
                nc.sync.dma_start(out=ostack[64:128, :], in_=tmpB[0:64, :])
                nc.vector.tensor_tensor(
                    out=ostack[64:128, :],
                    in0=ostack[64:128, :],
                    in1=bc[64:128, :],
                    op=ALU.mult,
                )
                # out-projection for these 512 tokens, staged then one DMA
                ob_big = ob_pool.tile([128, 4, D], bf16)
                for t4 in range(4):
                    for Dc in range(2):
                        ps_op = ps_st.tile([128, 512], f32, tag="psS")
                        nc.tensor.matmul(
                            ps_op,
                            ostack[:, t4 * 128 : (t4 + 1) * 128],
                            wout_sb[:, Dc * 512 : (Dc + 1) * 512],
                            start=True,
                            stop=True,
                        )
                        nc.vector.tensor_copy(
                            out=ob_big[:, t4, Dc * 512 : (Dc + 1) * 512], in_=ps_op
                        )
                nc.sync.dma_start(
                    out=out_d[i0 : i0 + 512, :].rearrange(
                        "(t p) d -> p t d", p=128
                    ),
                    in_=ob_big,
                )

            for tt in range(4):
                phase_a(tt)
            for b in range(4):
                for it in range(4):
                    if b < 3:
                        tt_next = 4 * (b + 1) + it
                        phase_b_it(b, it, mid_emit=lambda tt=tt_next: phase_a(tt))
                    else:
                        phase_b_it(b, it)

    nc.finalize()
    return nc


def make_in_maps(x, ln_gamma, ln_beta, w_qkv):
    bf = _np_bf16()
    x = np.asarray(x, np.float32).reshape(TOK, D).astype(bf)
    g = np.asarray(ln_gamma, np.float32)
    w = np.asarray(w_qkv, np.float32)
    w_eff = g[:, None] * w  # [D, 3*INNER]

    # rope tables
    inv_freq = 1.0 / (10000.0 ** (np.arange(0, HD, 2, dtype=np.float32) / HD))
    pos = np.arange(N, dtype=np.float32)
    ang = pos[:, None] * inv_freq[None, :]  # [N, 32]
    cosT = np.cos(ang).T.astype(np.float32)  # [32, N]
    sinT = np.sin(ang).T.astype(np.float32)
    cos128 = np.tile(cosT, (4, 1)).astype(bf)  # rows p -> cos[p%32]
    sin128s = np.tile(sinT, (4, 1)).astype(np.float32)
    sin128s[0:32] *= -1.0
    sin128s[64:96] *= -1.0
    sin128s = sin128s.astype(bf)

    perm_np = np.zeros((128, 128), np.float32)
    for p in range(128):
        sig = (p % 64 + 32) % 64 + 64 * (p // 64)
        perm_np[sig, p] = 1.0
    perm_np = perm_np.astype(bf)

    e2_np = np.zeros((2, 128), np.float32)
    e2_np[0, 0:64] = 1.0
    e2_np[1, 64:128] = 1.0
    e2_np = e2_np.astype(bf)

    in_maps = []
    for c in range(NCORES):
        sl = slice(128 * c, 128 * c + 128)
        wq = w_eff[:, 0:1024][:, sl]
        wk = w_eff[:, 1024:2048][:, sl]
        wv = w_eff[:, 2048:3072][:, sl]

        def swap_halves(m):
            m4 = m.reshape(D, 2, 2, 32)
            return m4[:, :, ::-1, :].reshape(D, 128)

        wcat = np.concatenate([wq, wk, wv], axis=1).astype(bf)
        in_maps.append(
            {
                "x": x,
                "wqkv": np.ascontiguousarray(wcat),
                "wout": None,  # filled below by caller (needs w_out)
                "cos128": cos128,
                "sin128s": sin128s,
                "e2": e2_np,
                "perm": perm_np,
            }
        )
    return in_maps


def _run(inputs, trace=False):
    from concourse import bass_utils

    if "nc" not in _CACHE:
        _CACHE["nc"] = build_bass()
    nc = _CACHE["nc"]

    bf = _np_bf16()
    x = inputs["x"]
    w_out = np.asarray(inputs["w_out"], np.float32)
    b_out = np.asarray(inputs["b_out"], np.float32)
    beta = np.asarray(inputs["ln_beta"], np.float32)
    assert np.allclose(beta, 0.0, atol=1e-12), "nonzero ln_beta unsupported"

    in_maps = make_in_maps(
        inputs["x"], inputs["ln_gamma"], inputs["ln_beta"], inputs["w_qkv"]
    )
    for c in range(NCORES):
        in_maps[c]["wout"] = np.ascontiguousarray(
            w_out[128 * c : 128 * c + 128, :].astype(bf)
        )

    res = bass_utils.run_bass_kernel_spmd(
        nc, in_maps, core_ids=list(range(NCORES)), trace=trace
    )
    total = np.zeros((TOK, D), np.float32)
    for r in res.results:
        total += np.asarray(r["out"], np.float32)
    total += b_out[None, :]
    return total.reshape(B, N, D), res


def kernel(**inputs):
    out, _ = _run(inputs, trace=False)
    return out


# revision 8
# speedup vs baseline: 2.3631x; 1.0043x over previous
"""Trainium2 8-core kernel for LN + RoPE multi-head attention + out-proj.

Sharding: tensor-parallel over heads. Core c owns heads (2c, 2c+1) = inner dims
[128c, 128c+128). Each core computes LN(x) @ its w_qkv column-slice, RoPE,
full-sequence attention for its 2 heads, and a partial out-projection against
its w_out row-slice. Host sums the 8 partial outputs and adds b_out.

Math notes:
- ln_gamma is folded into w_qkv on the host (w_eff = gamma[:,None] * w_qkv);
  ln_beta contributes beta @ w_qkv which is zero for this problem (beta = 0).
- RoPE is computed as q' = q*cos128 + q_swap*sin128s where q_swap comes from an
  extra matmul against half-rotated weight columns (avoids cross-partition ops)
  and sin128s carries the rotation signs.
- Softmax denominators come from a constant ones-column appended to v, so the
  attention matmul accumulates sum(exp) in psum row 64 for free; normalization
  happens on the small per-head output, broadcast across partitions via a tiny
  K=2 matmul.

v2 performance structure:
- Phase B is software-pipelined around the ACT(exp) stream, which is the
  per-iteration floor ((172+1024)/1.2GHz ~ 1.1us per 128x1024 fp32-psum tile).
  Scores pairs for jc+2 are emitted before the probs@v matmuls of jc so the PE
  queue never head-of-line blocks on an exp.
- Elementwise work is spread across DVE / ACT / GPSIMD so no engine exceeds
  the PE total: rope mul/add on gpsimd, psum->sbuf casts split ACT/DVE.
- Phase A tiles for batch b+1 are interleaved between phase-B attention loops
  of batch b, emitted before the normalize/out-proj tail so their PE work
  slots under batch b's exp stream.
"""

import os
import sys

sys.path.insert(0, "/opt/trn_rl_repo")

import numpy as np

B, N, D = 4, 2048, 1024
H, HD = 16, 64
TOK = B * N  # 8192
NCORES = 8
EPS = 1e-5

_CACHE = {}


def _np_bf16():
    import ml_dtypes

    return ml_dtypes.bfloat16


def build_bass():
    import concourse.bass as bass
    import concourse.mybir as mybir
    import concourse.tile as tile
    from concourse import bacc
    from concourse.masks import make_identity

    f32 = mybir.dt.float32
    bf16 = mybir.dt.bfloat16
    AF = mybir.ActivationFunctionType
    ALU = mybir.AluOpType

    # Force Ln and Exp to resolve to the one table set containing both, so the
    # scheduler never ping-pongs ACT table loads between LN and softmax phases.
    import concourse.hw_specs as hw_specs
    if not getattr(hw_specs, "_ln_exp_patched", False):
        _orig_gat = hw_specs.get_activation_tables

        def _patched_gat(arch):
            tabs = _orig_gat(arch)
            AFt = mybir.ActivationFunctionType
            for name, funcs in tabs.items():
                if name != "natural_log_exp_and_others":
                    funcs.discard(AFt.Exp)
                    funcs.discard(AFt.Ln)
            return tabs

        hw_specs.get_activation_tables = _patched_gat
        import concourse.bacc as _bacc_mod
        _bacc_mod.get_activation_tables = _patched_gat
        hw_specs._ln_exp_patched = True

    nc = bacc.Bacc("TRN2", target_bir_lowering=False, debug=False, num_devices=NCORES)

    x_d = nc.dram_tensor("x", [TOK, D], bf16, kind="ExternalInput").ap()
    wqkv_d = nc.dram_tensor("wqkv", [D, 384], bf16, kind="ExternalInput").ap()
    perm_d = nc.dram_tensor("perm", [128, 128], bf16, kind="ExternalInput").ap()
    wout_d = nc.dram_tensor("wout", [128, D], bf16, kind="ExternalInput").ap()
    cos_d = nc.dram_tensor("cos128", [128, N], bf16, kind="ExternalInput").ap()
    sin_d = nc.dram_tensor("sin128s", [128, N], bf16, kind="ExternalInput").ap()
    e2_d = nc.dram_tensor("e2", [2, 128], bf16, kind="ExternalInput").ap()
    out_d = nc.dram_tensor("out", [TOK, D], bf16, kind="ExternalOutput").ap()

    with tile.TileContext(nc) as tc:
        with (
            tc.tile_pool(name="singles", bufs=1) as singles,
            tc.tile_pool(name="xin", bufs=2) as xin_pool,
            tc.tile_pool(name="xn", bufs=3) as xn_pool,
            tc.tile_pool(name="xnt", bufs=3) as xnt_pool,
            tc.tile_pool(name="small", bufs=6) as small,
            tc.tile_pool(name="rtmp", bufs=6) as rtmp,
            tc.tile_pool(name="vst", bufs=3) as vst_pool,
            tc.tile_pool(name="probs", bufs=3) as pr_pool,
            tc.tile_pool(name="tmph", bufs=4) as tmph_pool,
            tc.tile_pool(name="norm", bufs=2) as norm_pool,
            tc.tile_pool(name="ost", bufs=3) as ost_pool,
            tc.tile_pool(name="ob", bufs=2) as ob_pool,
            tc.tile_pool(name="psA", bufs=2, space="PSUM") as ps_a,
            tc.tile_pool(name="psS", bufs=3, space="PSUM") as ps_st,
            tc.tile_pool(name="psOS", bufs=3, space="PSUM") as ps_os_pool,
        ):
            # ---- constants / persistent tiles ----
            ident = singles.tile([128, 128], bf16)
            make_identity(nc, ident)
            eps_sb = singles.tile([128, 1], f32)
            nc.vector.memset(eps_sb, EPS)
            e2 = singles.tile([2, 128], bf16)
            nc.sync.dma_start(out=e2, in_=e2_d)

            wqkv_sb = singles.tile([128, 8, 384], bf16)
            perm_sb = singles.tile([128, 128], bf16)
            nc.sync.dma_start(out=perm_sb, in_=perm_d)
            nc.sync.dma_start(
                out=wqkv_sb, in_=wqkv_d.rearrange("(c p) f -> p c f", p=128)
            )
            wout_sb = singles.tile([128, D], bf16)
            nc.sync.dma_start(out=wout_sb, in_=wout_d)
            cos_sb = singles.tile([128, N], bf16)
            nc.sync.dma_start(out=cos_sb, in_=cos_d)
            sin_sb = singles.tile([128, N], bf16)
            nc.sync.dma_start(out=sin_sb, in_=sin_d)

            qT = singles.tile([128, TOK], bf16)  # rows: head0 dims 0-63, head1 64-127
            kT = singles.tile([128, TOK], bf16)
            v_sb = singles.tile([128, 64, 2, 65], bf16)  # [j, jchunk, head, 64v+1]
            nc.vector.memset(v_sb[:, :, :, 64:65], 1.0)

            # ---- phase A: LN -> transpose -> QKV(+swapped) -> RoPE ----
            def phase_a(tt):  # 512-token tiles
                p0 = (tt % 4) * 512  # position within batch for rope tables
                xnT = xnt_pool.tile([128, 8, 512], bf16)
                x_t4 = xin_pool.tile([128, 4, D], bf16)
                nc.gpsimd.dma_start(
                    out=x_t4,
                    in_=x_d[tt * 512 : (tt + 1) * 512, :].rearrange(
                        "(t p) d -> p t d", p=128
                    ),
                )
                for st in range(4):
                    x_t = x_t4[:, st, :]
                    stats = small.tile([128, 2, 6], f32)
                    nc.vector.bn_stats(out=stats[:, 0, :], in_=x_t[:, 0:512])
                    nc.vector.bn_stats(out=stats[:, 1, :], in_=x_t[:, 512:1024])
                    mv = small.tile([128, 2], f32)
                    nc.vector.bn_aggr(out=mv, in_=stats)
                    lnv = small.tile([128, 1], f32)
                    nc.scalar.activation(lnv, mv[:, 1:2], AF.Ln, bias=eps_sb)
                    rstd = small.tile([128, 1], f32)
                    nc.scalar.activation(rstd, lnv, AF.Exp, scale=-0.5)
                    xn = xn_pool.tile([128, D], bf16)
                    nc.vector.tensor_scalar(
                        out=xn,
                        in0=x_t,
                        scalar1=mv[:, 0:1],
                        scalar2=rstd,
                        op0=ALU.subtract,
                        op1=ALU.mult,
                    )
                    # 8 transposes into one psum tile, one strided copy out
                    ptx = ps_a.tile([128, 1024], bf16, tag="psA")
                    for dc in range(8):
                        nc.tensor.transpose(
                            ptx[:, dc * 128 : (dc + 1) * 128],
                            xn[:, dc * 128 : (dc + 1) * 128],
                            ident,
                        )
                    nc.vector.tensor_copy(
                        out=xnT[:, :, st * 128 : st * 128 + 128],
                        in_=ptx.rearrange("p (c t) -> p c t", c=8),
                    )
                # QKV projections: f= 0:q 1:k 2:v
                for f in (0, 1, 2):
                    ps_q = ps_a.tile([128, 512], f32, tag="psA")
                    for dc in range(8):
                        nc.tensor.matmul(
                            ps_q,
                            wqkv_sb[:, dc, f * 128 : (f + 1) * 128],
                            xnT[:, dc, :],
                            start=dc == 0,
                            stop=dc == 7,
                        )
                    if f == 2:
                        # v: transpose to token-major into v_sb
                        vstage = vst_pool.tile([128, 512], bf16)
                        nc.vector.tensor_copy(out=vstage, in_=ps_q)
                        ptv = ps_a.tile([128, 512], bf16, tag="psA")
                        for st in range(4):
                            nc.tensor.transpose(
                                ptv[:, st * 128 : (st + 1) * 128],
                                vstage[:, st * 128 : (st + 1) * 128],
                                ident,
                            )
                        nc.vector.tensor_copy(
                            out=v_sb[:, tt * 4 : tt * 4 + 4, :, 0:64],
                            in_=ptv.rearrange("p (c h d) -> p c h d", c=4, h=2),
                        )
                    else:
                        # rope: qT = q*cos + (perm.T @ q)*sin_signed
                        q_sb = rtmp.tile([128, 512], bf16, tag="qsb")
                        nc.scalar.activation(out=q_sb, in_=ps_q, func=AF.Copy)
                        ps_qsw = ps_a.tile([128, 512], f32, tag="psA")
                        nc.tensor.matmul(
                            ps_qsw, perm_sb, q_sb, start=True, stop=True
                        )
                        a = rtmp.tile([128, 512], bf16, tag="ra")
                        nc.vector.tensor_tensor(
                            out=a, in0=q_sb, in1=cos_sb[:, p0 : p0 + 512], op=ALU.mult
                        )
                        bt = rtmp.tile([128, 512], bf16, tag="rb")
                        nc.vector.tensor_tensor(
                            out=bt, in0=ps_qsw, in1=sin_sb[:, p0 : p0 + 512], op=ALU.mult
                        )
                        dst = qT if f == 0 else kT
                        nc.vector.tensor_tensor(
                            out=dst[:, tt * 512 : (tt + 1) * 512],
                            in0=a,
                            in1=bt,
                            op=ALU.add,
                        )

            # ---- phase B: scores -> softmax -> probs@v -> normalize -> out ----
            def emit_attn(b, it):
                """Scores/exp/probs@v stream for one 512-query tile; returns the
                two per-head psum accumulators (64 rows out + 1 row sum-exp)."""
                i0 = b * 2048 + it * 512
                ps_os = [
                    ps_os_pool.tile([65, 512], f32, tag="psOS", name=f"ps_o_{b}_{it}_{h}")
                    for h in range(2)
                ]
                probs_tiles = {}

                def emit_scores(jc):
                    j0 = b * 2048 + jc * 128
                    ps_h = []
                    for h in range(2):
                        hb = h * 64
                        ps_s = ps_st.tile([128, 512], f32, tag="psS")
                        nc.tensor.matmul(
                            ps_s,
                            kT[hb : hb + 64, j0 : j0 + 128],
                            qT[hb : hb + 64, i0 : i0 + 512],
                            start=True,
                            stop=True,
                            tile_position=(hb, 0),
                        )
                        ps_h.append(ps_s)
                    probs = pr_pool.tile([128, 2, 512], bf16)
                    for h in range(2):
                        nc.scalar.activation(
                            probs[:, h, :], ps_h[h], AF.Exp, scale=HD**-0.5
                        )
                    probs_tiles[jc] = probs

                def emit_v(jc):
                    jcg = b * 16 + jc
                    probs = probs_tiles.pop(jc)
                    for h in range(2):
                        nc.tensor.matmul(
                            ps_os[h],
                            v_sb[:, jcg, h, :],
                            probs[:, h, :],
                            start=jc == 0,
                            stop=jc == 15,
                        )

                PIPE = 2
                for jc in range(PIPE):
                    emit_scores(jc)
                for jc in range(16):
                    if jc + PIPE < 16:
                        emit_scores(jc + PIPE)
                    emit_v(jc)
                return ps_os

            def emit_tail(b, it, ps_os):
                """Normalize by sum-exp and out-project one 512-query tile."""
                i0 = b * 2048 + it * 512
                tmpA = tmph_pool.tile([65, 512], bf16, tag="tmpA")
                nc.vector.tensor_copy(out=tmpA, in_=ps_os[0])
                tmpB = tmph_pool.tile([65, 512], bf16, tag="tmpB")
                nc.vector.tensor_copy(out=tmpB, in_=ps_os[1])
                # denominators -> reciprocal -> broadcast via K=2 matmul
                rbf = norm_pool.tile([2, 512], bf16, tag="rbf")
                nc.sync.dma_start(out=rbf[0:1, :], in_=tmpA[64:65, :])
                nc.sync.dma_start(out=rbf[1:2, :], in_=tmpB[64:65, :])
                rf = norm_pool.tile([2, 512], f32, tag="rf")
                nc.vector.tensor_copy(out=rf, in_=rbf)
                nc.vector.reciprocal(out=rf, in_=rf)
                rbf2 = norm_pool.tile([2, 512], bf16, tag="rbf2")
                nc.vector.tensor_copy(out=rbf2, in_=rf)
                ps_bc = ps_st.tile([128, 512], f32, tag="psS")
                nc.tensor.matmul(ps_bc, e2, rbf2, start=True, stop=True)
                bc = norm_pool.tile([128, 512], bf16, tag="bc")
                nc.scalar.activation(out=bc, in_=ps_bc, func=AF.Copy)
                ostack = ost_pool.tile([128, 512], bf16)
                nc.vector.tensor_tensor(
                    out=ostack[0:64, :], in0=tmpA[0:64, :], in1=bc[0:64, :], op=ALU.mult
                )
                nc.sync.dma_start(out=ostack[64:128, :], in_=tmpB[0:64, :])
                nc.vector.tensor_tensor(
                    out=ostack[64:128, :],
                    in0=ostack[64:128, :],
                    in1=bc[64:128, :],
                    op=ALU.mult,
                )
                # out-projection for these 512 tokens, staged then one DMA
                ob_big = ob_pool.tile([128, 4, D], bf16)
                for t4 in range(4):
                    for Dc in range(2):
                        ps_op = ps_st.tile([128, 512], f32, tag="psS")
                        nc.tensor.matmul(
                            ps_op,
                            ostack[:, t4 * 128 : (t4 + 1) * 128],
                            wout_sb[:, Dc * 512 : (Dc + 1) * 512],
                            start=True,
                            stop=True,
                        )
                        nc.vector.tensor_copy(
                            out=ob_big[:, t4, Dc * 512 : (Dc + 1) * 512], in_=ps_op
                        )
                nc.sync.dma_start(
                    out=out_d[i0 : i0 + 512, :].rearrange(
                        "(t p) d -> p t d", p=128
                    ),
                    in_=ob_big,
                )

            # Flat software-pipelined stream over the 16 (b, it) units: the
            # normalize/out-proj tail of unit u-1 and the phase-A tile for
            # batch b+1 are emitted inside unit u's exp window, so the PE/DVE
            # work they carry overlaps the ACT-bound attention stream instead
            # of stalling it.
            for tt in range(4):
                phase_a(tt)
            pending = None  # (b, it, ps_os) awaiting tail emission
            for u, (b, it) in enumerate((b, it) for b in range(4) for it in range(4)):
                ps_os = emit_attn(b, it)
                if pending is not None:
                    emit_tail(*pending)
                if u < 12:
                    phase_a(4 + u)
                pending = (b, it, ps_os)
            emit_tail(*pending)

    nc.finalize()
    return nc


def make_in_maps(x, ln_gamma, ln_beta, w_qkv):
    bf = _np_bf16()
    x = np.asarray(x, np.float32).reshape(TOK, D).astype(bf)
    g = np.asarray(ln_gamma, np.float32)
    w = np.asarray(w_qkv, np.float32)
    w_eff = g[:, None] * w  # [D, 3*INNER]

    # rope tables
    inv_freq = 1.0 / (10000.0 ** (np.arange(0, HD, 2, dtype=np.float32) / HD))
    pos = np.arange(N, dtype=np.float32)
    ang = pos[:, None] * inv_freq[None, :]  # [N, 32]
    cosT = np.cos(ang).T.astype(np.float32)  # [32, N]
    sinT = np.sin(ang).T.astype(np.float32)
    cos128 = np.tile(cosT, (4, 1)).astype(bf)  # rows p -> cos[p%32]
    sin128s = np.tile(sinT, (4, 1)).astype(np.float32)
    sin128s[0:32] *= -1.0
    sin128s[64:96] *= -1.0
    sin128s = sin128s.astype(bf)

    perm_np = np.zeros((128, 128), np.float32)
    for p in range(128):
        sig = (p % 64 + 32) % 64 + 64 * (p // 64)
        perm_np[sig, p] = 1.0
    perm_np = perm_np.astype(bf)

    e2_np = np.zeros((2, 128), np.float32)
    e2_np[0, 0:64] = 1.0
    e2_np[1, 64:128] = 1.0
    e2_np = e2_np.astype(bf)

    in_maps = []
    for c in range(NCORES):
        sl = slice(128 * c, 128 * c + 128)
        wq = w_eff[:, 0:1024][:, sl]
        wk = w_eff[:, 1024:2048][:, sl]
        wv = w_eff[:, 2048:3072][:, sl]

        def swap_halves(m):
            m4 = m.reshape(D, 2, 2, 32)
            return m4[:, :, ::-1, :].reshape(D, 128)

        wcat = np.concatenate([wq, wk, wv], axis=1).astype(bf)
        in_maps.append(
            {
                "x": x,
                "wqkv": np.ascontiguousarray(wcat),
                "wout": None,  # filled below by caller (needs w_out)
                "cos128": cos128,
                "sin128s": sin128s,
                "e2": e2_np,
                "perm": perm_np,
            }
        )
    return in_maps


def _run(inputs, trace=False):
    from concourse import bass_utils

    if "nc" not in _CACHE:
        _CACHE["nc"] = build_bass()
    nc = _CACHE["nc"]

    bf = _np_bf16()
    x = inputs["x"]
    w_out = np.asarray(inputs["w_out"], np.float32)
    b_out = np.asarray(inputs["b_out"], np.float32)
    beta = np.asarray(inputs["ln_beta"], np.float32)
    assert np.allclose(beta, 0.0, atol=1e-12), "nonzero ln_beta unsupported"

    in_maps = make_in_maps(
        inputs["x"], inputs["ln_gamma"], inputs["ln_beta"], inputs["w_qkv"]
    )
    for c in range(NCORES):
        in_maps[c]["wout"] = np.ascontiguousarray(
            w_out[128 * c : 128 * c + 128, :].astype(bf)
        )

    res = bass_utils.run_bass_kernel_spmd(
        nc, in_maps, core_ids=list(range(NCORES)), trace=trace
    )
    total = np.zeros((TOK, D), np.float32)
    for r in res.results:
        total += np.asarray(r["out"], np.float32)
    total += b_out[None, :]
    return total.reshape(B, N, D), res


def kernel(**inputs):
    out, _ = _run(inputs, trace=False)
    return out


# revision 9
# speedup vs baseline: 2.8206x; 1.1936x over previous
"""Trainium2 8-core kernel for LN + RoPE multi-head attention + out-proj.

Sharding: tensor-parallel over heads. Core c owns heads (2c, 2c+1) = inner dims
[128c, 128c+128). Each core computes LN(x) @ its w_qkv column-slice, RoPE,
full-sequence attention for its 2 heads, and a partial out-projection against
its w_out row-slice. Host sums the 8 partial outputs and adds b_out.

Math notes:
- ln_gamma is folded into w_qkv on the host (w_eff = gamma[:,None] * w_qkv);
  ln_beta contributes beta @ w_qkv which is zero for this problem (beta = 0).
- RoPE is computed as q' = q*cos128 + q_swap*sin128s where q_swap comes from an
  extra matmul against half-rotated weight columns (avoids cross-partition ops)
  and sin128s carries the rotation signs.
- Softmax denominators come from a constant ones-column appended to v, so the
  attention matmul accumulates sum(exp) in psum row 64 for free; normalization
  happens on the small per-head output, broadcast across partitions via a tiny
  K=2 matmul.

v2 performance structure:
- Phase B is software-pipelined around the ACT(exp) stream, which is the
  per-iteration floor ((172+1024)/1.2GHz ~ 1.1us per 128x1024 fp32-psum tile).
  Scores pairs for jc+2 are emitted before the probs@v matmuls of jc so the PE
  queue never head-of-line blocks on an exp.
- Elementwise work is spread across DVE / ACT / GPSIMD so no engine exceeds
  the PE total: rope mul/add on gpsimd, psum->sbuf casts split ACT/DVE.
- Phase A tiles for batch b+1 are interleaved between phase-B attention loops
  of batch b, emitted before the normalize/out-proj tail so their PE work
  slots under batch b's exp stream.
"""

import os
import sys

sys.path.insert(0, "/opt/trn_rl_repo")

import numpy as np

B, N, D = 4, 2048, 1024
H, HD = 16, 64
TOK = B * N  # 8192
NCORES = 8
EPS = 1e-5

_CACHE = {}


def _np_bf16():
    import ml_dtypes

    return ml_dtypes.bfloat16


def build_bass():
    import concourse.bass as bass
    import concourse.mybir as mybir
    import concourse.tile as tile
    from concourse import bacc
    from concourse.masks import make_identity

    f32 = mybir.dt.float32
    bf16 = mybir.dt.bfloat16
    AF = mybir.ActivationFunctionType
    ALU = mybir.AluOpType

    # Force Ln and Exp to resolve to the one table set containing both, so the
    # scheduler never ping-pongs ACT table loads between LN and softmax phases.
    import concourse.hw_specs as hw_specs
    if not getattr(hw_specs, "_ln_exp_patched", False):
        _orig_gat = hw_specs.get_activation_tables

        def _patched_gat(arch):
            tabs = _orig_gat(arch)
            AFt = mybir.ActivationFunctionType
            for name, funcs in tabs.items():
                if name != "natural_log_exp_and_others":
                    funcs.discard(AFt.Exp)
                    funcs.discard(AFt.Ln)
            return tabs

        hw_specs.get_activation_tables = _patched_gat
        import concourse.bacc as _bacc_mod
        _bacc_mod.get_activation_tables = _patched_gat
        hw_specs._ln_exp_patched = True

    nc = bacc.Bacc("TRN2", target_bir_lowering=False, debug=False, num_devices=NCORES)

    x_d = nc.dram_tensor("x", [TOK, D], bf16, kind="ExternalInput").ap()
    wqkv_d = nc.dram_tensor("wqkv", [D, 384], bf16, kind="ExternalInput").ap()
    perm_d = nc.dram_tensor("perm", [128, 128], bf16, kind="ExternalInput").ap()
    wout_d = nc.dram_tensor("wout", [128, D], bf16, kind="ExternalInput").ap()
    cos_d = nc.dram_tensor("cos128", [128, N], bf16, kind="ExternalInput").ap()
    sin_d = nc.dram_tensor("sin128s", [128, N], bf16, kind="ExternalInput").ap()
    e2_d = nc.dram_tensor("e2", [2, 128], bf16, kind="ExternalInput").ap()
    out_d = nc.dram_tensor("out", [TOK, D], bf16, kind="ExternalOutput").ap()

    with tile.TileContext(nc) as tc:
        with (
            tc.tile_pool(name="singles", bufs=1) as singles,
            tc.tile_pool(name="xin", bufs=2) as xin_pool,
            tc.tile_pool(name="xn", bufs=3) as xn_pool,
            tc.tile_pool(name="xnt", bufs=3) as xnt_pool,
            tc.tile_pool(name="small", bufs=6) as small,
            tc.tile_pool(name="rtmp", bufs=6) as rtmp,
            tc.tile_pool(name="vst", bufs=3) as vst_pool,
            tc.tile_pool(name="probs", bufs=3) as pr_pool,
            tc.tile_pool(name="tmph", bufs=4) as tmph_pool,
            tc.tile_pool(name="norm", bufs=2) as norm_pool,
            tc.tile_pool(name="ost", bufs=3) as ost_pool,
            tc.tile_pool(name="ob", bufs=2) as ob_pool,
            tc.tile_pool(name="psA", bufs=2, space="PSUM") as ps_a,
            tc.tile_pool(name="psS", bufs=2, space="PSUM") as ps_st,
            tc.tile_pool(name="psOS", bufs=2, space="PSUM") as ps_os_pool,
        ):
            # ---- constants / persistent tiles ----
            ident = singles.tile([128, 128], bf16)
            make_identity(nc, ident)
            eps_sb = singles.tile([128, 1], f32)
            nc.vector.memset(eps_sb, EPS)
            e2 = singles.tile([2, 128], bf16)
            nc.sync.dma_start(out=e2, in_=e2_d)

            wqkv_sb = singles.tile([128, 8, 384], bf16)
            perm_sb = singles.tile([128, 128], bf16)
            nc.sync.dma_start(out=perm_sb, in_=perm_d)
            nc.sync.dma_start(
                out=wqkv_sb, in_=wqkv_d.rearrange("(c p) f -> p c f", p=128)
            )
            wout_sb = singles.tile([128, D], bf16)
            nc.sync.dma_start(out=wout_sb, in_=wout_d)
            cos_sb = singles.tile([128, N], bf16)
            nc.sync.dma_start(out=cos_sb, in_=cos_d)
            sin_sb = singles.tile([128, N], bf16)
            nc.sync.dma_start(out=sin_sb, in_=sin_d)

            qT = singles.tile([128, TOK], bf16)  # rows: head0 dims 0-63, head1 64-127
            kT = singles.tile([128, TOK], bf16)
            v_sb = singles.tile([128, 64, 2, 65], bf16)  # [j, jchunk, head, 64v+1]
            nc.vector.memset(v_sb[:, :, :, 64:65], 1.0)

            # ---- phase A: LN -> transpose -> QKV(+swapped) -> RoPE ----
            def phase_a(tt):  # 512-token tiles
                p0 = (tt % 4) * 512  # position within batch for rope tables
                xnT = xnt_pool.tile([128, 8, 512], bf16)
                x_t4 = xin_pool.tile([128, 4, D], bf16)
                nc.gpsimd.dma_start(
                    out=x_t4,
                    in_=x_d[tt * 512 : (tt + 1) * 512, :].rearrange(
                        "(t p) d -> p t d", p=128
                    ),
                )
                for st in range(4):
                    x_t = x_t4[:, st, :]
                    stats = small.tile([128, 2, 6], f32)
                    nc.vector.bn_stats(out=stats[:, 0, :], in_=x_t[:, 0:512])
                    nc.vector.bn_stats(out=stats[:, 1, :], in_=x_t[:, 512:1024])
                    mv = small.tile([128, 2], f32)
                    nc.vector.bn_aggr(out=mv, in_=stats)
                    lnv = small.tile([128, 1], f32)
                    nc.scalar.activation(lnv, mv[:, 1:2], AF.Ln, bias=eps_sb)
                    rstd = small.tile([128, 1], f32)
                    nc.scalar.activation(rstd, lnv, AF.Exp, scale=-0.5)
                    xn = xn_pool.tile([128, D], bf16)
                    nc.vector.tensor_scalar(
                        out=xn,
                        in0=x_t,
                        scalar1=mv[:, 0:1],
                        scalar2=rstd,
                        op0=ALU.subtract,
                        op1=ALU.mult,
                    )
                    # 8 transposes into one psum tile, one strided copy out
                    ptx = ps_a.tile([128, 1024], bf16, tag="psA")
                    for dc in range(8):
                        nc.tensor.transpose(
                            ptx[:, dc * 128 : (dc + 1) * 128],
                            xn[:, dc * 128 : (dc + 1) * 128],
                            ident,
                        )
                    nc.vector.tensor_copy(
                        out=xnT[:, :, st * 128 : st * 128 + 128],
                        in_=ptx.rearrange("p (c t) -> p c t", c=8),
                    )
                # QKV projections: f= 0:q 1:k 2:v
                for f in (0, 1, 2):
                    ps_q = ps_a.tile([128, 512], f32, tag="psA")
                    for dc in range(8):
                        nc.tensor.matmul(
                            ps_q,
                            wqkv_sb[:, dc, f * 128 : (f + 1) * 128],
                            xnT[:, dc, :],
                            start=dc == 0,
                            stop=dc == 7,
                        )
                    if f == 2:
                        # v: transpose to token-major into v_sb
                        vstage = vst_pool.tile([128, 512], bf16)
                        nc.vector.tensor_copy(out=vstage, in_=ps_q)
                        ptv = ps_a.tile([128, 512], bf16, tag="psA")
                        for st in range(4):
                            nc.tensor.transpose(
                                ptv[:, st * 128 : (st + 1) * 128],
                                vstage[:, st * 128 : (st + 1) * 128],
                                ident,
                            )
                        nc.vector.tensor_copy(
                            out=v_sb[:, tt * 4 : tt * 4 + 4, :, 0:64],
                            in_=ptv.rearrange("p (c h d) -> p c h d", c=4, h=2),
                        )
                    else:
                        # rope: qT = q*cos + (perm.T @ q)*sin_signed
                        q_sb = rtmp.tile([128, 512], bf16, tag="qsb")
                        nc.scalar.activation(out=q_sb, in_=ps_q, func=AF.Copy)
                        ps_qsw = ps_a.tile([128, 512], f32, tag="psA")
                        nc.tensor.matmul(
                            ps_qsw, perm_sb, q_sb, start=True, stop=True
                        )
                        a = rtmp.tile([128, 512], bf16, tag="ra")
                        nc.vector.tensor_tensor(
                            out=a, in0=q_sb, in1=cos_sb[:, p0 : p0 + 512], op=ALU.mult
                        )
                        bt = rtmp.tile([128, 512], bf16, tag="rb")
                        nc.vector.tensor_tensor(
                            out=bt, in0=ps_qsw, in1=sin_sb[:, p0 : p0 + 512], op=ALU.mult
                        )
                        dst = qT if f == 0 else kT
                        nc.vector.tensor_tensor(
                            out=dst[:, tt * 512 : (tt + 1) * 512],
                            in0=a,
                            in1=bt,
                            op=ALU.add,
                        )

            # ---- phase B: scores -> softmax -> probs@v -> normalize -> out ----
            def emit_attn(b, it):
                """Scores/exp/probs@v stream for one 512-query tile; returns the
                two per-head psum accumulators (64 rows out + 1 row sum-exp)."""
                i0 = b * 2048 + it * 512
                ps_os = [
                    ps_os_pool.tile([65, 512], f32, tag="psOS", name=f"ps_o_{b}_{it}_{h}")
                    for h in range(2)
                ]
                probs_tiles = {}

                def emit_scores(jc):
                    j0 = b * 2048 + jc * 128
                    ps_s = ps_st.tile([128, 1024], f32, tag="psS")
                    for h in range(2):
                        hb = h * 64
                        nc.tensor.matmul(
                            ps_s[:, h * 512 : (h + 1) * 512],
                            kT[hb : hb + 64, j0 : j0 + 128],
                            qT[hb : hb + 64, i0 : i0 + 512],
                            start=True,
                            stop=True,
                            tile_position=(hb, 0),
                        )
                    probs = pr_pool.tile([128, 1024], bf16)
                    nc.scalar.activation(probs, ps_s, AF.Exp, scale=HD**-0.5)
                    probs_tiles[jc] = probs

                def emit_v(jc):
                    jcg = b * 16 + jc
                    probs = probs_tiles.pop(jc)
                    for h in range(2):
                        nc.tensor.matmul(
                            ps_os[h],
                            v_sb[:, jcg, h, :],
                            probs[:, h * 512 : (h + 1) * 512],
                            start=jc == 0,
                            stop=jc == 15,
                        )

                PIPE = 2
                for jc in range(PIPE):
                    emit_scores(jc)
                for jc in range(16):
                    if jc + PIPE < 16:
                        emit_scores(jc + PIPE)
                    emit_v(jc)
                return ps_os

            def emit_tail(b, it, ps_os):
                """Normalize by sum-exp and out-project one 512-query tile."""
                i0 = b * 2048 + it * 512
                tmpA = tmph_pool.tile([65, 512], bf16, tag="tmpA")
                nc.vector.tensor_copy(out=tmpA, in_=ps_os[0])
                tmpB = tmph_pool.tile([65, 512], bf16, tag="tmpB")
                nc.vector.tensor_copy(out=tmpB, in_=ps_os[1])
                # denominators -> reciprocal -> broadcast via K=2 matmul
                rbf = norm_pool.tile([2, 512], bf16, tag="rbf")
                nc.sync.dma_start(out=rbf[0:1, :], in_=tmpA[64:65, :])
                nc.sync.dma_start(out=rbf[1:2, :], in_=tmpB[64:65, :])
                rf = norm_pool.tile([2, 512], f32, tag="rf")
                nc.vector.tensor_copy(out=rf, in_=rbf)
                nc.vector.reciprocal(out=rf, in_=rf)
                rbf2 = norm_pool.tile([2, 512], bf16, tag="rbf2")
                nc.vector.tensor_copy(out=rbf2, in_=rf)
                ps_bc = ps_st.tile([128, 512], f32, tag="psS")
                nc.tensor.matmul(ps_bc, e2, rbf2, start=True, stop=True)
                bc = norm_pool.tile([128, 512], bf16, tag="bc")
                nc.scalar.activation(out=bc, in_=ps_bc, func=AF.Copy)
                ostack = ost_pool.tile([128, 512], bf16)
                nc.vector.tensor_tensor(
                    out=ostack[0:64, :], in0=tmpA[0:64, :], in1=bc[0:64, :], op=ALU.mult
                )
                nc.sync.dma_start(out=ostack[64:128, :], in_=tmpB[0:64, :])
                nc.vector.tensor_tensor(
                    out=ostack[64:128, :],
                    in0=ostack[64:128, :],
                    in1=bc[64:128, :],
                    op=ALU.mult,
                )
                # out-projection for these 512 tokens, staged then one DMA
                ob_big = ob_pool.tile([128, 4, D], bf16)
                for t4 in range(4):
                    for Dc in range(2):
                        ps_op = ps_st.tile([128, 512], f32, tag="psS")
                        nc.tensor.matmul(
                            ps_op,
                            ostack[:, t4 * 128 : (t4 + 1) * 128],
                            wout_sb[:, Dc * 512 : (Dc + 1) * 512],
                            start=True,
                            stop=True,
                        )
                        nc.vector.tensor_copy(
                            out=ob_big[:, t4, Dc * 512 : (Dc + 1) * 512], in_=ps_op
                        )
                nc.sync.dma_start(
                    out=out_d[i0 : i0 + 512, :].rearrange(
                        "(t p) d -> p t d", p=128
                    ),
                    in_=ob_big,
                )

            # Flat software-pipelined stream over the 16 (b, it) units: the
            # normalize/out-proj tail of unit u-1 and the phase-A tile for
            # batch b+1 are emitted inside unit u's exp window, so the PE/DVE
            # work they carry overlaps the ACT-bound attention stream instead
            # of stalling it.
            for tt in range(4):
                phase_a(tt)
            pending = None  # (b, it, ps_os) awaiting tail emission
            for u, (b, it) in enumerate((b, it) for b in range(4) for it in range(4)):
                ps_os = emit_attn(b, it)
                if pending is not None:
                    emit_tail(*pending)
                if u < 12:
                    phase_a(4 + u)
                pending = (b, it, ps_os)
            emit_tail(*pending)

    nc.finalize()
    return nc


def make_in_maps(x, ln_gamma, ln_beta, w_qkv):
    bf = _np_bf16()
    x = np.asarray(x, np.float32).reshape(TOK, D).astype(bf)
    g = np.asarray(ln_gamma, np.float32)
    w = np.asarray(w_qkv, np.float32)
    w_eff = g[:, None] * w  # [D, 3*INNER]

    # rope tables
    inv_freq = 1.0 / (10000.0 ** (np.arange(0, HD, 2, dtype=np.float32) / HD))
    pos = np.arange(N, dtype=np.float32)
    ang = pos[:, None] * inv_freq[None, :]  # [N, 32]
    cosT = np.cos(ang).T.astype(np.float32)  # [32, N]
    sinT = np.sin(ang).T.astype(np.float32)
    cos128 = np.tile(cosT, (4, 1)).astype(bf)  # rows p -> cos[p%32]
    sin128s = np.tile(sinT, (4, 1)).astype(np.float32)
    sin128s[0:32] *= -1.0
    sin128s[64:96] *= -1.0
    sin128s = sin128s.astype(bf)

    perm_np = np.zeros((128, 128), np.float32)
    for p in range(128):
        sig = (p % 64 + 32) % 64 + 64 * (p // 64)
        perm_np[sig, p] = 1.0
    perm_np = perm_np.astype(bf)

    e2_np = np.zeros((2, 128), np.float32)
    e2_np[0, 0:64] = 1.0
    e2_np[1, 64:128] = 1.0
    e2_np = e2_np.astype(bf)

    in_maps = []
    for c in range(NCORES):
        sl = slice(128 * c, 128 * c + 128)
        wq = w_eff[:, 0:1024][:, sl]
        wk = w_eff[:, 1024:2048][:, sl]
        wv = w_eff[:, 2048:3072][:, sl]

        def swap_halves(m):
            m4 = m.reshape(D, 2, 2, 32)
            return m4[:, :, ::-1, :].reshape(D, 128)

        wcat = np.concatenate([wq, wk, wv], axis=1).astype(bf)
        in_maps.append(
            {
                "x": x,
                "wqkv": np.ascontiguousarray(wcat),
                "wout": None,  # filled below by caller (needs w_out)
                "cos128": cos128,
                "sin128s": sin128s,
                "e2": e2_np,
                "perm": perm_np,
            }
        )
    return in_maps


def _run(inputs, trace=False):
    from concourse import bass_utils

    if "nc" not in _CACHE:
        _CACHE["nc"] = build_bass()
    nc = _CACHE["nc"]

    bf = _np_bf16()
    x = inputs["x"]
    w_out = np.asarray(inputs["w_out"], np.float32)
    b_out = np.asarray(inputs["b_out"], np.float32)
    beta = np.asarray(inputs["ln_beta"], np.float32)
    assert np.allclose(beta, 0.0, atol=1e-12), "nonzero ln_beta unsupported"

    in_maps = make_in_maps(
        inputs["x"], inputs["ln_gamma"], inputs["ln_beta"], inputs["w_qkv"]
    )
    for c in range(NCORES):
        in_maps[c]["wout"] = np.ascontiguousarray(
            w_out[128 * c : 128 * c + 128, :].astype(bf)
        )

    res = bass_utils.run_bass_kernel_spmd(
        nc, in_maps, core_ids=list(range(NCORES)), trace=trace
    )
    total = np.zeros((TOK, D), np.float32)
    for r in res.results:
        total += np.asarray(r["out"], np.float32)
    total += b_out[None, :]
    return total.reshape(B, N, D), res


def kernel(**inputs):
    out, _ = _run(inputs, trace=False)
    return out
